# revision 1
# baseline (speedup 1.0000x reference)
"""Masked (expander) linear layer on 8 Trainium2 NeuronCores.

Computes out = x @ (W * M)^T for
  x: [16384, 2048] f32, W: [2048, 2048] f32, M: [2048, 2048] int32 (0/1)

Sharding: pure data-parallel over rows of x. Each of the 8 cores gets 2048
rows of x plus a replicated (transposed) copy of W and M, computes its
[2048, 2048] output shard entirely locally (mask-multiply on DVE, matmul on
PE), and the host concatenates shards. No collectives.

Device-side design:
 - All tensors are laid out on host so the contraction dim lands on SBUF
   partitions: W and M transposed panel-major ([NT, IN, 512], contiguous
   panels), x transposed per core ([IN, rows]). Layout-only host prep;
   every FLOP of the module (mask multiply + matmul) runs on device. The
   mask is passed as int8 (0/1, lossless repack) to cut DMA traffic.
 - Matmuls run in float32r mode (single-pass fp32_mode=HIGH PE streaming,
   1 cycle/row, vs 4 cycles/row for plain fp32; 1.35e-4 rel err at
   K=2048). The walrus verifier requires f32r operands to come from
   f32r-rounding producers: the DVE mask-multiply writes wm as f32r and
   x streams in through SWDGE cast-DMAs (f32 -> f32r).
 - Each DMA ring carries one stream so per-DMA fixed costs overlap:
   W 1MB k-quarter pieces on the sync HWDGE ring (3-deep staging
   pipeline), masks on the scalar ring, x on the SWDGE ring, outputs on
   the scalar ring. wm is stored as one tile per (n-chunk, k-quarter) so
   matmul sub-groups depend only on their own piece - PE starts ~17us in
   and stays fed through the whole weight load (keeps HAM at full clock).
 - m-tiles are processed in blocks of 4, n-chunk outer inside a block;
   x quarter-tiles are single-buffered and re-streamed just-in-time (the
   next block's quarter q loads right behind this block's last reader of
   quarter q). PSUM groups rotate over 8 banks with evacuation (ScalarE
   copy + DMA) inlined right after each group closes. (PSUM groups
   rotate over all 8 banks.)
"""

from contextlib import ExitStack

import numpy as np

import concourse.bacc as bacc
import concourse.bass as bass
import concourse.mybir as mybir
import concourse.tile as tile
from concourse.bass_utils import run_bass_kernel_spmd

N_CORES = 8
P = 128

FULL_N, FULL_OUT, FULL_IN = 16384, 2048, 2048

MASK_DTYPES = {
    "int8": (mybir.dt.int8, np.int8),
    "int32": (mybir.dt.int32, np.int32),
    "float32": (mybir.dt.float32, np.float32),
}


def build_nc(
    rows: int = FULL_N // N_CORES,
    in_dim: int = FULL_IN,
    out_dim: int = FULL_OUT,
    mm_dtype=mybir.dt.float32r,
    mask_dtype: str = "int8",
    n_chunk: int = 512,
    m_block: int = 4,
):
    """Per-core Bass module: y[rows, out] = x @ (wt * m).

    DRAM layouts: wt/mk panel-major [NT, in_dim, n_chunk]; x transposed
    [in_dim, rows]; y row-major [rows, out_dim].
    """
    assert rows % P == 0 and in_dim % P == 0 and out_dim % n_chunk == 0
    KT = in_dim // P
    MT = rows // P
    NT = out_dim // n_chunk
    assert KT % 4 == 0 and MT % m_block == 0
    KQ = KT // 4
    NB = MT // m_block
    mw = m_block * P  # columns of x per block

    mdt, _ = MASK_DTYPES[mask_dtype]

    nc = bacc.Bacc("TRN2", target_bir_lowering=False, debug=False)
    x = nc.dram_tensor("x", [in_dim, rows], mybir.dt.float32, kind="ExternalInput")
    wt = nc.dram_tensor(
        "wt", [NT, in_dim, n_chunk], mybir.dt.float32, kind="ExternalInput"
    )
    mk = nc.dram_tensor("mk", [NT, in_dim, n_chunk], mdt, kind="ExternalInput")
    y = nc.dram_tensor("y", [rows, out_dim], mybir.dt.float32, kind="ExternalOutput")

    # K-major DRAM views: [.., p, kt, ..]
    wt_v = wt[:, :, :].rearrange("t (kt p) n -> t p kt n", p=P)
    mk_v = mk[:, :, :].rearrange("t (kt p) n -> t p kt n", p=P)
    x_v = x[:, :].rearrange("(kt p) m -> p kt m", p=P)

    with ExitStack() as ctx:
        tc = ctx.enter_context(tile.TileContext(nc))
        wm_pool = ctx.enter_context(tc.tile_pool(name="wm", bufs=1))
        ws_pool = ctx.enter_context(tc.tile_pool(name="ws", bufs=3))
        msk_pool = ctx.enter_context(tc.tile_pool(name="msk", bufs=3))
        xt_pool = ctx.enter_context(tc.tile_pool(name="xt", bufs=1))
        yo_pool = ctx.enter_context(tc.tile_pool(name="yo", bufs=3))
        pm_pool = ctx.enter_context(tc.tile_pool(name="pm", bufs=1, space="PSUM"))

        # Resident masked weight: wm_t[nt][q] of shape [P, KQ, n_chunk]
        wm_t = [
            [
                wm_pool.tile(
                    [P, KQ, n_chunk], mm_dtype, tag=f"wm{nt}_{q}", name=f"wm{nt}_{q}"
                )
                for q in range(4)
            ]
            for nt in range(NT)
        ]
        # x tiles: [P, KQ, m_block*P] per k-quarter (single set; the next
        # block's quarter q streams in right after this block's last reader
        # of quarter q)
        xt_t = [
            xt_pool.tile([P, KQ, mw], mm_dtype, tag=f"xt{q}", name=f"xt{q}")
            for q in range(4)
        ]

        def load_w_piece(nt, q):
            ksl = slice(q * KQ, (q + 1) * KQ)
            # W rides the sync HWDGE ring alone (own per-DMA fixed costs)
            wstage = ws_pool.tile([P, KQ, n_chunk], mybir.dt.float32, tag="ws")
            nc.sync.dma_start(out=wstage[:], in_=wt_v[nt, :, ksl, :])
            # masks ride the scalar ring (done before output stores begin)
            mtile = msk_pool.tile([P, KQ, n_chunk], mdt, tag="mt")
            nc.scalar.dma_start(out=mtile[:], in_=mk_v[nt, :, ksl, :])
            for k in range(KQ):
                # masked multiply; DVE f32r output is the rounding producer
                nc.vector.tensor_mul(
                    wm_t[nt][q][:, k, :], wstage[:, k, :], mtile[:, k, :]
                )

        def load_x_piece(b, q):
            ksl = slice(q * KQ, (q + 1) * KQ)
            # SWDGE cast-DMA f32 -> f32r (the rounding producer); x has the
            # SWDGE ring to itself. Two m-half DMAs: the first half's WAR
            # clears as soon as mb 0/1 finish reading, so the JIT re-stream
            # at block boundaries starts (and lands) earlier.
            hw = mw // 2
            for h in range(2):
                nc.gpsimd.dma_start(
                    out=xt_t[q][:, :, h * hw : (h + 1) * hw],
                    in_=x_v[:, ksl, b * mw + h * hw : b * mw + (h + 1) * hw],
                )

        # ---- prep: x block 0 on the SWDGE ring, W pieces on sync ----
        for q in range(4):
            load_x_piece(0, q)
        for nt in range(NT):
            for q in range(4):
                load_w_piece(nt, q)

        # ---- main: blocks of m_block m-tiles; nt-outer inside a block ----
        for b in range(NB):
            xts = xt_t
            for nt in range(NT):
                # 6 rotating PSUM banks: group g frees its bank 6 groups later
                pms = {
                    mb: pm_pool.tile(
                        [P, n_chunk],
                        mybir.dt.float32,
                        tag=f"pm{(nt * m_block + mb) % 8}",
                        name=f"pm{(nt * m_block + mb) % 8}",
                    )
                    for mb in range(m_block)
                }
                # k-quarter-outer: each sub-group only needs its own pieces
                for q in range(4):
                    for mb in range(m_block):
                        for k in range(KQ):
                            kt = q * KQ + k
                            nc.tensor.matmul(
                                pms[mb][:],
                                xts[q][:, k, bass.ts(mb, P)],
                                wm_t[nt][q][:, k, :],
                                start=(kt == 0),
                                stop=(kt == KT - 1),
                            )
                        if q == 3:
                            # evacuate as soon as this group closes
                            mt = b * m_block + mb
                            yo = yo_pool.tile(
                                [P, n_chunk], mybir.dt.float32, tag="yo"
                            )
                            nc.scalar.copy(yo[:], pms[mb][:])
                            nc.scalar.dma_start(
                                out=y[mt * P : (mt + 1) * P, bass.ts(nt, n_chunk)],
                                in_=yo[:],
                            )
                    if nt == NT - 1 and b + 1 < NB:
                        # last reader of x quarter q just finished; stream in
                        # the next block's quarter q behind it
                        load_x_piece(b + 1, q)

    nc.compile()
    return nc


def _prep_host(input_, weight, mask, mask_dtype="int8", n_chunk=512):
    _, npdt = MASK_DTYPES[mask_dtype]
    in_dim, out_dim = weight.shape[1], weight.shape[0]
    nt = out_dim // n_chunk
    # weight.T -> [NT, IN, n_chunk], each panel contiguous
    wtp = np.ascontiguousarray(weight.T.reshape(in_dim, nt, n_chunk).transpose(1, 0, 2))
    mkp = np.ascontiguousarray(
        mask.T.reshape(in_dim, nt, n_chunk).transpose(1, 0, 2)
    ).astype(npdt)
    rows = input_.shape[0] // N_CORES
    in_maps = []
    for c in range(N_CORES):
        xp = np.ascontiguousarray(input_[c * rows : (c + 1) * rows].T)
        in_maps.append({"x": xp, "wt": wtp, "mk": mkp})
    return in_maps


_CACHE = {}


def _run(input_, weight, mask, trace=False, **build_kw):
    rows_total, in_dim = input_.shape
    out_dim = weight.shape[0]
    key = (rows_total, in_dim, out_dim, tuple(sorted(build_kw.items())))
    if key not in _CACHE:
        _CACHE[key] = build_nc(
            rows=rows_total // N_CORES, in_dim=in_dim, out_dim=out_dim, **build_kw
        )
    nc = _CACHE[key]
    in_maps = _prep_host(
        input_,
        weight,
        mask,
        build_kw.get("mask_dtype", "int8"),
        build_kw.get("n_chunk", 512),
    )
    res = run_bass_kernel_spmd(nc, in_maps, core_ids=list(range(N_CORES)), trace=trace)
    out = np.concatenate([res.results[c]["y"] for c in range(N_CORES)], axis=0)
    return out, res


def kernel(input_, weight, mask):
    input_ = np.asarray(input_, dtype=np.float32)
    weight = np.asarray(weight, dtype=np.float32)
    mask = np.asarray(mask)
    out, _ = _run(input_, weight, mask, trace=False)
    return out



# revision 2
# speedup vs baseline: 1.0918x; 1.0918x over previous
"""Masked (expander) linear layer on 8 Trainium2 NeuronCores.

Computes out = x @ (W * M)^T for
  x: [16384, 2048] f32, W: [2048, 2048] f32, M: [2048, 2048] int32 (0/1)

Sharding: pure data-parallel over rows of x. Each of the 8 cores gets 2048
rows of x plus a replicated (transposed) copy of W and M, computes its
[2048, 2048] output shard entirely locally (mask-multiply on DVE, matmul on
PE), and the host concatenates shards. No collectives.

Device-side design:
 - All tensors are laid out on host so the contraction dim lands on SBUF
   partitions: W and M transposed panel-major ([NT, IN, 512], contiguous
   panels), x transposed per core ([IN, rows]). Layout-only host prep;
   every FLOP of the module (mask multiply + matmul) runs on device. The
   mask is passed as int8 (0/1, lossless repack) to cut DMA traffic.
 - Matmuls run in float32r mode (single-pass fp32_mode=HIGH PE streaming,
   1 cycle/row, vs 4 cycles/row for plain fp32; 1.35e-4 rel err at
   K=2048). The walrus verifier requires f32r operands to come from
   f32r-rounding producers: the DVE mask-multiply writes wm as f32r and
   x streams in through SWDGE cast-DMAs (f32 -> f32r).
 - Each DMA ring carries one stream so per-DMA fixed costs overlap:
   W 1MB k-quarter pieces on the sync HWDGE ring (3-deep staging
   pipeline), masks on the scalar ring, x on the SWDGE ring, outputs on
   the scalar ring. wm is stored as one tile per (n-chunk, k-quarter) so
   matmul sub-groups depend only on their own piece - PE starts ~17us in
   and stays fed through the whole weight load (keeps HAM at full clock).
 - m-tiles are processed in blocks of 4, n-chunk outer inside a block;
   x quarter-tiles are single-buffered and re-streamed just-in-time (the
   next block's quarter q loads right behind this block's last reader of
   quarter q). PSUM groups rotate over 8 banks with evacuation (ScalarE
   copy + DMA) inlined right after each group closes. (PSUM groups
   rotate over all 8 banks.)
"""

from contextlib import ExitStack

import numpy as np

import concourse.bacc as bacc
import concourse.bass as bass
import concourse.mybir as mybir
import concourse.tile as tile
from concourse.bass_utils import run_bass_kernel_spmd

N_CORES = 8
P = 128

FULL_N, FULL_OUT, FULL_IN = 16384, 2048, 2048

MASK_DTYPES = {
    "int8": (mybir.dt.int8, np.int8),
    "int32": (mybir.dt.int32, np.int32),
    "float32": (mybir.dt.float32, np.float32),
}


def build_nc(
    rows: int = FULL_N // N_CORES,
    in_dim: int = FULL_IN,
    out_dim: int = FULL_OUT,
    mm_dtype=mybir.dt.bfloat16,
    mask_dtype: str = "int8",
    n_chunk: int = 512,
    m_block: int = 8,
):
    """Per-core Bass module: y[rows, out] = x @ (wt * m).

    DRAM layouts: wt/mk panel-major [NT, in_dim, n_chunk]; x transposed
    [in_dim, rows]; y row-major [rows, out_dim].
    """
    assert rows % P == 0 and in_dim % P == 0 and out_dim % n_chunk == 0
    KT = in_dim // P
    MT = rows // P
    NT = out_dim // n_chunk
    assert KT % 4 == 0 and MT % m_block == 0
    KQ = KT // 4
    NB = MT // m_block
    mw = m_block * P  # columns of x per block

    mdt, _ = MASK_DTYPES[mask_dtype]

    nc = bacc.Bacc("TRN2", target_bir_lowering=False, debug=False)
    x = nc.dram_tensor("x", [in_dim, rows], mybir.dt.float32, kind="ExternalInput")
    wt = nc.dram_tensor(
        "wt", [NT, in_dim, n_chunk], mybir.dt.float32, kind="ExternalInput"
    )
    mk = nc.dram_tensor("mk", [NT, in_dim, n_chunk], mdt, kind="ExternalInput")
    y = nc.dram_tensor("y", [rows, out_dim], mybir.dt.float32, kind="ExternalOutput")

    # K-major DRAM views: [.., p, kt, ..]
    wt_v = wt[:, :, :].rearrange("t (kt p) n -> t p kt n", p=P)
    mk_v = mk[:, :, :].rearrange("t (kt p) n -> t p kt n", p=P)
    x_v = x[:, :].rearrange("(kt p) m -> p kt m", p=P)

    with ExitStack() as ctx:
        tc = ctx.enter_context(tile.TileContext(nc))
        wm_pool = ctx.enter_context(tc.tile_pool(name="wm", bufs=1))
        ws_pool = ctx.enter_context(tc.tile_pool(name="ws", bufs=3))
        msk_pool = ctx.enter_context(tc.tile_pool(name="msk", bufs=3))
        xt_pool = ctx.enter_context(tc.tile_pool(name="xt", bufs=1))
        yo_pool = ctx.enter_context(tc.tile_pool(name="yo", bufs=3))
        pm_pool = ctx.enter_context(tc.tile_pool(name="pm", bufs=1, space="PSUM"))

        # Resident masked weight: wm_t[nt][q] of shape [P, KQ, n_chunk]
        wm_t = [
            [
                wm_pool.tile(
                    [P, KQ, n_chunk], mm_dtype, tag=f"wm{nt}_{q}", name=f"wm{nt}_{q}"
                )
                for q in range(4)
            ]
            for nt in range(NT)
        ]
        # x tiles: [P, KQ, m_block*P] per k-quarter (single set; the next
        # block's quarter q streams in right after this block's last reader
        # of quarter q)
        xt_t = [
            xt_pool.tile([P, KQ, mw], mm_dtype, tag=f"xt{q}", name=f"xt{q}")
            for q in range(4)
        ]

        def load_w_piece(nt, q):
            ksl = slice(q * KQ, (q + 1) * KQ)
            # W rides the sync HWDGE ring alone (own per-DMA fixed costs)
            wstage = ws_pool.tile([P, KQ, n_chunk], mybir.dt.float32, tag="ws")
            nc.sync.dma_start(out=wstage[:], in_=wt_v[nt, :, ksl, :])
            # masks ride the scalar ring (done before output stores begin)
            mtile = msk_pool.tile([P, KQ, n_chunk], mdt, tag="mt")
            nc.scalar.dma_start(out=mtile[:], in_=mk_v[nt, :, ksl, :])
            for k in range(KQ):
                # masked multiply; DVE f32r output is the rounding producer
                nc.vector.tensor_mul(
                    wm_t[nt][q][:, k, :], wstage[:, k, :], mtile[:, k, :]
                )

        def load_x_piece(b, q):
            ksl = slice(q * KQ, (q + 1) * KQ)
            # SWDGE cast-DMA f32 -> f32r (the rounding producer); x has the
            # SWDGE ring to itself. Two m-half DMAs: the first half's WAR
            # clears as soon as mb 0/1 finish reading, so the JIT re-stream
            # at block boundaries starts (and lands) earlier.
            hw = mw // 2
            for h in range(2):
                nc.gpsimd.dma_start(
                    out=xt_t[q][:, :, h * hw : (h + 1) * hw],
                    in_=x_v[:, ksl, b * mw + h * hw : b * mw + (h + 1) * hw],
                )

        # ---- prep: x block 0 on the SWDGE ring, W pieces on sync ----
        for q in range(4):
            load_x_piece(0, q)
        for nt in range(NT):
            for q in range(4):
                load_w_piece(nt, q)

        # ---- main: blocks of m_block m-tiles; nt-outer inside a block ----
        for b in range(NB):
            xts = xt_t
            for nt in range(NT):
                # 6 rotating PSUM banks: group g frees its bank 6 groups later
                pms = {
                    mb: pm_pool.tile(
                        [P, n_chunk],
                        mybir.dt.float32,
                        tag=f"pm{(nt * m_block + mb) % 8}",
                        name=f"pm{(nt * m_block + mb) % 8}",
                    )
                    for mb in range(m_block)
                }
                # k-quarter-outer: each sub-group only needs its own pieces
                for q in range(4):
                    for mb in range(m_block):
                        for k in range(KQ):
                            kt = q * KQ + k
                            nc.tensor.matmul(
                                pms[mb][:],
                                xts[q][:, k, bass.ts(mb, P)],
                                wm_t[nt][q][:, k, :],
                                start=(kt == 0),
                                stop=(kt == KT - 1),
                            )
                        if q == 3:
                            # evacuate as soon as this group closes
                            mt = b * m_block + mb
                            yo = yo_pool.tile(
                                [P, n_chunk], mybir.dt.float32, tag="yo"
                            )
                            nc.scalar.copy(yo[:], pms[mb][:])
                            nc.scalar.dma_start(
                                out=y[mt * P : (mt + 1) * P, bass.ts(nt, n_chunk)],
                                in_=yo[:],
                            )
                    if nt == NT - 1 and b + 1 < NB:
                        # last reader of x quarter q just finished; stream in
                        # the next block's quarter q behind it
                        load_x_piece(b + 1, q)

    nc.compile()
    return nc


def _prep_host(input_, weight, mask, mask_dtype="int8", n_chunk=512):
    _, npdt = MASK_DTYPES[mask_dtype]
    in_dim, out_dim = weight.shape[1], weight.shape[0]
    nt = out_dim // n_chunk
    # weight.T -> [NT, IN, n_chunk], each panel contiguous
    wtp = np.ascontiguousarray(weight.T.reshape(in_dim, nt, n_chunk).transpose(1, 0, 2))
    mkp = np.ascontiguousarray(
        mask.T.reshape(in_dim, nt, n_chunk).transpose(1, 0, 2)
    ).astype(npdt)
    rows = input_.shape[0] // N_CORES
    in_maps = []
    for c in range(N_CORES):
        xp = np.ascontiguousarray(input_[c * rows : (c + 1) * rows].T)
        in_maps.append({"x": xp, "wt": wtp, "mk": mkp})
    return in_maps


_CACHE = {}


def _run(input_, weight, mask, trace=False, **build_kw):
    rows_total, in_dim = input_.shape
    out_dim = weight.shape[0]
    key = (rows_total, in_dim, out_dim, tuple(sorted(build_kw.items())))
    if key not in _CACHE:
        _CACHE[key] = build_nc(
            rows=rows_total // N_CORES, in_dim=in_dim, out_dim=out_dim, **build_kw
        )
    nc = _CACHE[key]
    in_maps = _prep_host(
        input_,
        weight,
        mask,
        build_kw.get("mask_dtype", "int8"),
        build_kw.get("n_chunk", 512),
    )
    res = run_bass_kernel_spmd(nc, in_maps, core_ids=list(range(N_CORES)), trace=trace)
    out = np.concatenate([res.results[c]["y"] for c in range(N_CORES)], axis=0)
    return out, res


def kernel(input_, weight, mask):
    input_ = np.asarray(input_, dtype=np.float32)
    weight = np.asarray(weight, dtype=np.float32)
    mask = np.asarray(mask)
    out, _ = _run(input_, weight, mask, trace=False)
    return out



# revision 4
# speedup vs baseline: 1.1384x; 1.0428x over previous
"""Masked (expander) linear layer on 8 Trainium2 NeuronCores.

Computes out = x @ (W * M)^T for
  x: [16384, 2048] f32, W: [2048, 2048] f32, M: [2048, 2048] int32 (0/1)

Sharding: pure data-parallel over rows of x. Each of the 8 cores gets 2048
rows of x plus a replicated (transposed) copy of W and M, computes its
[2048, 2048] output shard entirely locally (mask-multiply on DVE, matmul on
PE), and the host concatenates shards. No collectives.

Device-side design (v3, bf16):
 - All matmuls run in bf16 (1 PE cycle/row — same peak as f32r — but
   LDWEIGHTS gets Fast-Weight-Load, ~100ns vs ~227ns for fp32, so the
   weight loads fully hide behind the 512-cycle moving stream; measured
   MATMUL spacing hits the 216ns streaming floor). PSUM accumulates f32
   over the full K=2048, outputs stored f32. Measured rel err ~2.2e-3.
 - Transport is bf16/int8: host pre-packs x and W to bf16 (bit-identical
   to the previous on-device cast-DMA, since mask is 0/1 the DVE
   mask-multiply result is the same) halving ramp HBM traffic; mask rides
   as int8. All module arithmetic (mask multiply, matmul) stays on device.
 - Ramp: W/mask stream piece-by-piece (one piece = 4 k-tiles x 512 n) on
   the sync/scalar rings; m-tiles are processed in blocks of 8 so each
   arriving piece unlocks 32 matmuls (~7us) vs ~2.2us piece delivery --
   the PE never starves and HAM stays at full clock.
 - x is double-buffered per block (2 sets of [P, KQ, 1024] bf16 tiles);
   block b+1's x streams on the gpsimd ring while block b computes,
   gated behind block b's nt>=2 so it can't compete with the W ramp.
 - PSUM groups rotate over all 8 banks (8 live groups per nt); each group
   closes after its 16-kt accumulation and is evacuated immediately
   (copy alternates ScalarE/VectorE to balance engines, then DMA out on
   the sync/scalar rings alternately).
"""

from contextlib import ExitStack

import numpy as np

import concourse.bacc as bacc
import concourse.bass as bass
import concourse.mybir as mybir
import concourse.tile as tile
from concourse.bass_utils import run_bass_kernel_spmd

N_CORES = 8
P = 128

FULL_N, FULL_OUT, FULL_IN = 16384, 2048, 2048


def build_nc(
    rows: int = FULL_N // N_CORES,
    in_dim: int = FULL_IN,
    out_dim: int = FULL_OUT,
    n_chunk: int = 512,
    m_block: int = 8,
):
    """Per-core Bass module: y[rows, out] = x @ (wt * m).

    DRAM layouts: wt/mk panel-major [NT, in_dim, n_chunk] (wt bf16, mk int8);
    x transposed bf16 [in_dim, rows]; y row-major f32 [rows, out_dim].
    """
    assert rows % P == 0 and in_dim % P == 0 and out_dim % n_chunk == 0
    KT = in_dim // P
    MT = rows // P
    NT = out_dim // n_chunk
    assert KT % 4 == 0 and MT % m_block == 0
    KQ = KT // 4
    NB = MT // m_block
    mw = m_block * P  # columns of x per block

    bf16 = mybir.dt.bfloat16

    nc = bacc.Bacc("TRN2", target_bir_lowering=False, debug=False)
    x = nc.dram_tensor("x", [in_dim, rows], bf16, kind="ExternalInput")
    wt = nc.dram_tensor("wt", [NT, in_dim, n_chunk], bf16, kind="ExternalInput")
    mk = nc.dram_tensor("mk", [NT, in_dim, n_chunk], mybir.dt.int8, kind="ExternalInput")
    y = nc.dram_tensor("y", [rows, out_dim], mybir.dt.float32, kind="ExternalOutput")

    # K-major DRAM views: [.., p, kt, ..]
    wt_v = wt[:, :, :].rearrange("t (kt p) n -> t p kt n", p=P)
    mk_v = mk[:, :, :].rearrange("t (kt p) n -> t p kt n", p=P)
    x_v = x[:, :].rearrange("(kt p) m -> p kt m", p=P)

    with ExitStack() as ctx:
        tc = ctx.enter_context(tile.TileContext(nc))
        wm_pool = ctx.enter_context(tc.tile_pool(name="wm", bufs=1))
        ws_pool = ctx.enter_context(tc.tile_pool(name="ws", bufs=3))
        msk_pool = ctx.enter_context(tc.tile_pool(name="msk", bufs=3))
        xt_pool = ctx.enter_context(tc.tile_pool(name="xt", bufs=1))
        yo_pool = ctx.enter_context(tc.tile_pool(name="yo", bufs=4))
        pm_pool = ctx.enter_context(tc.tile_pool(name="pm", bufs=1, space="PSUM"))

        # Resident masked weight: wm_t[nt][q] of shape [P, KQ, n_chunk]
        wm_t = [
            [
                wm_pool.tile([P, KQ, n_chunk], bf16, tag=f"wm{nt}_{q}", name=f"wm{nt}_{q}")
                for q in range(4)
            ]
            for nt in range(NT)
        ]
        # x tiles: double-buffered per block parity: [set][q] -> [P, KQ, mw]
        xt_t = [
            [
                xt_pool.tile([P, KQ, mw], bf16, tag=f"xt{s}_{q}", name=f"xt{s}_{q}")
                for q in range(4)
            ]
            for s in range(2)
        ]

        def load_w_piece(nt, q):
            ksl = slice(q * KQ, (q + 1) * KQ)
            # W rides the sync HWDGE ring alone during the ramp
            wstage = ws_pool.tile([P, KQ, n_chunk], bf16, tag="ws")
            nc.sync.dma_start(out=wstage[:], in_=wt_v[nt, :, ksl, :])
            # masks ride the scalar ring
            mtile = msk_pool.tile([P, KQ, n_chunk], mybir.dt.int8, tag="mt")
            nc.scalar.dma_start(out=mtile[:], in_=mk_v[nt, :, ksl, :])
            for k in range(KQ):
                # masked multiply on DVE (bf16 x int8 -> bf16)
                nc.vector.tensor_mul(
                    wm_t[nt][q][:, k, :], wstage[:, k, :], mtile[:, k, :]
                )

        def load_x_piece(b, q):
            ksl = slice(q * KQ, (q + 1) * KQ)
            # x (pre-cast bf16 on host) on the gpsimd SWDGE ring; two m-half
            # DMAs so the first half lands (and unblocks MMs) earlier.
            hw = mw // 2
            xt = xt_t[b % 2][q]
            for h in range(2):
                nc.gpsimd.dma_start(
                    out=xt[:, :, h * hw : (h + 1) * hw],
                    in_=x_v[:, ksl, b * mw + h * hw : b * mw + (h + 1) * hw],
                )

        # ---- prep: x block 0 on the SWDGE ring, W pieces on sync/scalar ----
        for q in range(4):
            load_x_piece(0, q)
        for nt in range(NT):
            for q in range(4):
                load_w_piece(nt, q)

        # ---- main: blocks of m_block m-tiles; nt-outer inside a block ----
        evac_i = 0
        for b in range(NB):
            xts = xt_t[b % 2]
            for nt in range(NT):
                # 8 rotating PSUM banks: group (nt, mb) lives on bank mb
                pms = {
                    mb: pm_pool.tile(
                        [P, n_chunk],
                        mybir.dt.float32,
                        tag=f"pm{(nt * m_block + mb) % 8}",
                        name=f"pm{(nt * m_block + mb) % 8}",
                    )
                    for mb in range(m_block)
                }
                # k-quarter-outer: each sub-group only needs its own pieces
                for q in range(4):
                    for mb in range(m_block):
                        for k in range(KQ):
                            kt = q * KQ + k
                            nc.tensor.matmul(
                                pms[mb][:],
                                xts[q][:, k, bass.ts(mb, P)],
                                wm_t[nt][q][:, k, :],
                                start=(kt == 0),
                                stop=(kt == KT - 1),
                            )
                        if q == 3:
                            # evacuate as soon as this group closes; alternate
                            # engines so no single queue bottlenecks
                            mt = b * m_block + mb
                            yo = yo_pool.tile([P, n_chunk], mybir.dt.float32, tag="yo")
                            if evac_i % 2 == 0:
                                nc.scalar.copy(yo[:], pms[mb][:])
                            else:
                                nc.vector.tensor_copy(yo[:], pms[mb][:])
                            dma_eng = nc.sync if evac_i % 2 == 0 else nc.scalar
                            dma_eng.dma_start(
                                out=y[mt * P : (mt + 1) * P, bass.ts(nt, n_chunk)],
                                in_=yo[:],
                            )
                            evac_i += 1
                if nt == 2 and b + 1 < NB:
                    # W stream is done by now; prefetch next block's x into
                    # the other buffer set on the idle gpsimd ring
                    for q in range(4):
                        load_x_piece(b + 1, q)

    nc.compile()
    return nc


def _prep_host(input_, weight, mask, n_chunk=512):
    import ml_dtypes

    in_dim, out_dim = weight.shape[1], weight.shape[0]
    nt = out_dim // n_chunk
    # weight.T -> [NT, IN, n_chunk] bf16, each panel contiguous
    wtp = np.ascontiguousarray(
        weight.T.reshape(in_dim, nt, n_chunk).transpose(1, 0, 2)
    ).astype(ml_dtypes.bfloat16)
    mkp = np.ascontiguousarray(
        mask.T.reshape(in_dim, nt, n_chunk).transpose(1, 0, 2)
    ).astype(np.int8)
    rows = input_.shape[0] // N_CORES
    xbf = input_.astype(ml_dtypes.bfloat16)
    in_maps = []
    for c in range(N_CORES):
        xp = np.ascontiguousarray(xbf[c * rows : (c + 1) * rows].T)
        in_maps.append({"x": xp, "wt": wtp, "mk": mkp})
    return in_maps


_CACHE = {}


def _run(input_, weight, mask, trace=False, **build_kw):
    rows_total, in_dim = input_.shape
    out_dim = weight.shape[0]
    key = (rows_total, in_dim, out_dim, tuple(sorted(build_kw.items())))
    if key not in _CACHE:
        _CACHE[key] = build_nc(
            rows=rows_total // N_CORES, in_dim=in_dim, out_dim=out_dim, **build_kw
        )
    nc = _CACHE[key]
    in_maps = _prep_host(input_, weight, mask, build_kw.get("n_chunk", 512))
    res = run_bass_kernel_spmd(nc, in_maps, core_ids=list(range(N_CORES)), trace=trace)
    out = np.concatenate([res.results[c]["y"] for c in range(N_CORES)], axis=0)
    return out, res


def kernel(input_, weight, mask):
    input_ = np.asarray(input_, dtype=np.float32)
    weight = np.asarray(weight, dtype=np.float32)
    mask = np.asarray(mask)
    out, _ = _run(input_, weight, mask, trace=False)
    return out
